# revision 56
# baseline (speedup 1.0000x reference)
"""Trainium2 Bass kernel for nn_MhAttnBlock (GAT-style additive attention).

Reference computation (per batch b):
    Vproj = (V @ WV.T).reshape(k, H, 64)
    aK = K @ WK.T   (k, H)
    aQ = Q @ WQ.T   (q, H)
    w  = softmax_k(leaky_relu(aQ[q,h] + aK[k,h], 0.2))
    out[q, h*64+e] = sum_k w[q,k,h] * Vproj[k,h,e] + bias[h,e]

Algebra used on-device:
    exp(lrelu(s)) for s = aQ+aK equals max(A, B) = A + relu(B - A) with
       A = exp(aQ)*exp(aK)      (rank-1 in (q,k))
       B = exp(.2 aQ)*exp(.2 aK)
    The score grid D = B - A is a contraction-2 matmul from per-head exp
    row-vectors; relu(D) feeds a flash-style matmul against Vproj'
    (bias folded in, plus a ones column whose output is the softmax
    denominator); the rank-1 A-term is added via one small matmul from
    cv = sum_k eK1*Vp' kept in high precision (precision anchor).

Speed keys:
  - Q/K/V are fed to the device PRE-TRANSPOSED (host-side layout prep in
    make_inmaps, like the WVT/WQext weights) - no on-chip transposes.
  - Score D and flash run in fp8-e4m3 with MatmulPerfMode.DoubleRow
    (0.5 PE cycles/row).  Offline emulation on the exact inputs gives
    rel err ~1.04e-2 vs the 2e-2 budget.
  - Flash is q-major: lhsT = relu-grid tile (128,2,128q), rhs = Vp'
    (128,2,66) -> psum (128q, 66) at 33 cyc/mm, so no transpose epilogue.
  - relu (PSUM f32 -> SBUF fp8) is the elementwise bottleneck (Pool
    cannot read PSUM): split ACT/DVE ~36/28 and everything else is
    scheduled around keeping those two queues unblocked:
      * K/Q load first -> relus start ~14us while wv/V still stream in;
      * Vproj + cv issue INSIDE the grid loop at the position where
        their data arrives, so queued ops never block ready relus;
      * flash lags the relu pipeline (r8 ring buffers) and catches up
        two per iteration once Vp'/cv exist.

Sharding: data-parallel over batch B=8 across the 8 NeuronCores.
"""

import sys

for _p in ("/opt/trn_rl_repo", "/root/.axon_site/_ro/trn_rl_repo"):
    if _p not in sys.path:
        sys.path.insert(0, _p)

import numpy as np

import concourse.bass as bass  # noqa: F401
import concourse.bacc as bacc
import concourse.mybir as mybir
import concourse.tile as tile
from concourse.bass_utils import run_bass_kernel_spmd

F32 = mybir.dt.float32
F32R = mybir.dt.float32r
BF16 = mybir.dt.bfloat16
FP8 = mybir.dt.float8e4
AF = mybir.ActivationFunctionType
ALU = mybir.AluOpType
DR = mybir.MatmulPerfMode.DoubleRow

B, QS, KS = 8, 1024, 1024
D = 512          # qdim = kdim = vdim
H, OD = 8, 64    # heads, head out dim
NEG = 0.2
NCORES = 8

KT = KS // 128   # 8 k-tiles
QT = QS // 128   # 8 q-tiles
DT = D // 128    # 4 d-tiles
QB = QS // 512   # 2 q-blocks of 512
HB = OD + 2      # 66: [out 64 | den ones | pad]
DEN = OD         # ones/den column index within a head block
EH = 98          # padded head-group width for the cv matmul (cols 0/32/64/96)
LAG = 5          # flash lags relus by up to LAG grid iterations


def build_kernel():
    nc = bacc.Bacc()

    QTp = nc.declare_dram_parameter("QT", [D, QS], F32R, isOutput=False)
    KTp = nc.declare_dram_parameter("KT", [D, KS], F32R, isOutput=False)
    VTp = nc.declare_dram_parameter("VT", [D, KS], F32R, isOutput=False)
    # WQext/WKext: (D, 2H), cols 0..7 = W[h,:], cols 8..15 = 0.2*W[h,:]
    WQe = nc.declare_dram_parameter("WQext", [D, 2 * H], F32R, isOutput=False)
    WKe = nc.declare_dram_parameter("WKext", [D, 2 * H], F32R, isOutput=False)
    WVT = nc.declare_dram_parameter("WVT", [D, D], F32R, isOutput=False)
    # biasext: (1, H*HB): [bias[h,:64], 0, 0] per head
    BIA = nc.declare_dram_parameter("biasext", [1, H * HB], F32, isOutput=False)
    # sgn: (2H, 1): -1 on partitions 0..7 (negates eK1), +1 on 8..15
    SGN = nc.declare_dram_parameter("sgn", [2 * H, 1], F32, isOutput=False)
    OUT = nc.declare_dram_parameter("out", [QS, H * OD], F32, isOutput=True)

    with tile.TileContext(nc) as tc:
        with (
            tc.tile_pool(name="const", bufs=1) as constp,
            tc.tile_pool(name="stage", bufs=3) as stagep,
            tc.tile_pool(name="xt", bufs=1) as xtp,
            tc.tile_pool(name="big", bufs=1, side="right") as bigp,
            tc.tile_pool(name="r8p", bufs=28, side="right") as r8p,
        ):
            # ---- small constants (issued first on SP) ----
            sgn_sb = constp.tile([2 * H, 1], F32, tag="sgn")
            nc.sync.dma_start(sgn_sb[:], SGN[:])
            biasx = constp.tile([1, H * HB], F32, tag="biasx")
            nc.sync.dma_start(biasx[:], BIA[:])
            wq_sb = constp.tile([128, DT, 2 * H], F32R, tag="wq")
            nc.sync.dma_start(
                wq_sb[:], WQe.rearrange("(dt p) j -> p dt j", p=128)
            )
            wk_sb = constp.tile([128, DT, 2 * H], F32R, tag="wk")
            nc.sync.dma_start(
                wk_sb[:], WKe.rearrange("(dt p) j -> p dt j", p=128)
            )

            # ---- big tiles ----
            eQf = bigp.tile([2 * H, QS], F32, tag="eqf")
            eKf = bigp.tile([2 * H, KS], F32, tag="ekf")
            eq8p = bigp.tile([2 * H, QS], FP8, tag="eq8p")
            ek8p = bigp.tile([2 * H, KS], FP8, tag="ek8p")
            eq1p = bigp.tile([H, QS], BF16, tag="eq1p")
            # (a, h, q) order matches the (2H, KS) pre-stage rows so one DMA
            # moves the whole tile onto partition 0
            ek8 = bigp.tile([1, 2, H, KS], FP8, tag="ek8")
            eq8 = bigp.tile([1, 2, H, QS], FP8, tag="eq8")
            eq1f = bigp.tile([1, H * QS], BF16, tag="eq1f")
            eK1nS = bigp.tile([128, KT, 2, EH], F32R, tag="ek1ns")
            nc.gpsimd.memset(eK1nS[:].bitcast(F32), 0.0)
            vp_sb = bigp.tile([128, KT, H * HB], F32R, tag="vp")
            vp8 = bigp.tile([128, KT, H * HB], FP8, tag="vp8")
            cv_sb = constp.tile([1, H * HB], BF16, tag="cv")
            cvf = constp.tile([1, H * HB], F32, tag="cvf")
            biasbc = constp.tile([128, H * HB], F32, tag="biasbc")
            ones1 = constp.tile([1, 128], F32, tag="ones1")
            nc.vector.memset(ones1[:], 1.0)
            outF = bigp.tile([128, QT, H * OD], F32, tag="outf")

            # ---- activation loads: K and Q first (gate the relu stream),
            # wv/V later (consumed mid-grid) ----
            def load_xt(dram, tag, chunks):
                t_ = xtp.tile([128, DT, KS], F32R, tag=tag)
                view = dram.rearrange("(dt p) q -> p dt q", p=128)
                for lo, hi in chunks:
                    nc.sync.dma_start(t_[:, :, lo:hi], view[:, :, lo:hi])
                return t_

            kT = load_xt(KTp, "kT", [(0, KS)])
            qT = xtp.tile([128, DT, QS], F32R, tag="qT")
            qview = QTp.rearrange("(dt p) q -> p dt q", p=128)
            nc.sync.dma_start(qT[:, :, 0:512], qview[:, :, 0:512])
            nc.sync.dma_start(qT[:, :, 512:QS], qview[:, :, 512:QS])
            wv_sb = constp.tile([128, DT, D], F32R, tag="wv")
            nc.sync.dma_start(
                wv_sb[:], WVT.rearrange("(dt p) e -> p dt e", p=128)
            )
            vT = load_xt(VTp, "vT", [(0, 512), (512, KS)])

            # ---- psum pools: psd 3x2 banks + pso 1 + aux(shared) 1 = 8
            with (
                tc.tile_pool(name="psd", bufs=3, space="PSUM") as psdp,
                tc.tile_pool(name="pso", bufs=1, space="PSUM") as psop,
                tc.tile_pool(name="aux", bufs=1, space="PSUM") as auxp,
            ):
                # bias broadcast (also warms PE early)
                psb = auxp.tile([128, 512], F32, tag="aux")
                nc.tensor.matmul(
                    psb[:], lhsT=ones1[:], rhs=biasx[:, 0:512],
                    start=True, stop=True,
                )
                nc.vector.tensor_copy(out=biasbc[:, 0:512], in_=psb[:])
                psb = auxp.tile([128, 512], F32, tag="aux")
                nc.tensor.matmul(
                    psb[:, 0:H * HB - 512], lhsT=ones1[:],
                    rhs=biasx[:, 512:H * HB], start=True, stop=True,
                )
                nc.vector.tensor_copy(
                    out=biasbc[:, 512:H * HB],
                    in_=psb[:, 0:H * HB - 512],
                )

                # aKpair^T: exp, negate rows 0..7 (eK1), to fp8
                psk = auxp.tile([128, 512], F32, tag="aux")
                psk2 = auxp.tile([128, 512], F32, tag="aux")
                for half, pk in ((0, psk), (1, psk2)):
                    sl = slice(half * 512, (half + 1) * 512)
                    for dt in range(DT):
                        nc.tensor.matmul(
                            pk[0:2 * H, :],
                            lhsT=wk_sb[:, dt],
                            rhs=kT[:, dt, sl],
                            start=(dt == 0),
                            stop=(dt == DT - 1),
                        )
                    nc.scalar.activation(
                        eKf[:, sl], pk[0:2 * H, :], AF.Exp
                    )
                    nc.gpsimd.tensor_scalar(
                        out=ek8p[:, sl], in0=eKf[:, sl], scalar1=sgn_sb[:],
                        scalar2=None, op0=ALU.mult,
                    )
                nc.sync.dma_start(out=ek8[:], in_=ek8p[:])

                # aK natural (k-part, H) per k-tile -> eK1nS strided cols
                for t in range(KT):
                    psn = auxp.tile([128, 512], F32, tag="aux")
                    for dt in range(DT):
                        nc.tensor.matmul(
                            psn[:, 0:H],
                            lhsT=kT[:, dt, t * 128:(t + 1) * 128],
                            rhs=wk_sb[:, dt, 0:H],
                            start=(dt == 0),
                            stop=(dt == DT - 1),
                        )
                    nc.scalar.activation(
                        eK1nS[:, t, :, 0:EH:32],
                        psn[:, 0:H].rearrange("p (g i) -> p g i", g=2),
                        AF.Exp,
                    )

                # aQpair^T half 0 -> exp -> fp8 -> stage (scores for qb=0
                # need only this half)
                def qhalf(half):
                    sl = slice(half * 512, (half + 1) * 512)
                    pq = auxp.tile([128, 512], F32, tag="aux")
                    for dt in range(DT):
                        nc.tensor.matmul(
                            pq[0:2 * H, :],
                            lhsT=wq_sb[:, dt],
                            rhs=qT[:, dt, sl],
                            start=(dt == 0),
                            stop=(dt == DT - 1),
                        )
                    nc.scalar.activation(eQf[:, sl], pq[0:2 * H, :], AF.Exp)
                    nc.gpsimd.tensor_copy(out=eq8p[:, sl], in_=eQf[:, sl])
                    nc.sync.dma_start(
                        out=eq8[0:1, :, :, sl], in_=eq8p[:, sl]
                    )
                    nc.gpsimd.tensor_copy(
                        out=eq1p[:, sl], in_=eQf[0:H, sl]
                    )
                    nc.sync.dma_start(
                        out=eq1f[0:1].rearrange(
                            "o (h q) -> o h q", h=H
                        )[:, :, sl],
                        in_=eq1p[:, sl],
                    )

                qhalf(0)

                def vproj(trange):
                    # Vproj ktiles; vp8 = fp8(Vproj + bias) via DVE;
                    # vp_sb = f32r Vproj (no bias; cv gets a bias fixup)
                    for t in trange:
                        psv = auxp.tile([128, 512], F32, tag="aux")
                        for dt in range(DT):
                            nc.tensor.matmul(
                                psv[:],
                                lhsT=vT[:, dt, t * 128:(t + 1) * 128],
                                rhs=wv_sb[:, dt],
                                start=(dt == 0),
                                stop=(dt == DT - 1),
                            )
                        nc.vector.tensor_tensor(
                            out=vp8[:, t].rearrange(
                                "p (h e) -> p h e", h=H
                            )[:, :, 0:OD],
                            in0=psv[:].rearrange("p (h e) -> p h e", h=H),
                            in1=biasbc[:].rearrange(
                                "p (h e) -> p h e", h=H
                            )[:, :, 0:OD],
                            op=ALU.add,
                        )
                        nc.scalar.copy(
                            out=vp_sb[:, t].rearrange(
                                "p (h e) -> p h e", h=H
                            )[:, :, 0:OD],
                            in_=psv[:].rearrange("p (h e) -> p h e", h=H),
                        )

                def vcols_cv():
                    # den ones + zero pad columns; fp8 copy of them; then
                    # cv = sum_k eK1 * Vp (batched 4 heads, lhsT cols
                    # 0/32/64/96) with a sk*bias fixup into bf16 cv_sb
                    vpv = vp_sb[:].bitcast(F32).rearrange(
                        "p t (h e) -> p t h e", h=H
                    )
                    nc.gpsimd.memset(vpv[:, :, :, DEN:DEN + 1], 1.0)
                    nc.gpsimd.memset(vpv[:, :, :, DEN + 1:HB], 0.0)
                    nc.gpsimd.tensor_copy(
                        out=vp8[:].rearrange("p t (h e) -> p t h e", h=H)[
                            :, :, :, DEN:HB
                        ],
                        in_=vpv[:, :, :, DEN:HB],
                    )
                    for hg in range(2):
                        psc = auxp.tile([128, 512], F32, tag="aux")
                        for t in range(KT):
                            nc.tensor.matmul(
                                psc[0:EH, 0:4 * HB],
                                lhsT=eK1nS[:, t, hg],
                                rhs=vp_sb[:, t,
                                          hg * 4 * HB:(hg + 1) * 4 * HB],
                                start=(t == 0),
                                stop=(t == KT - 1),
                            )
                        for hi in range(4):
                            h = hg * 4 + hi
                            nc.vector.tensor_copy(
                                out=cvf[0:1, h * HB:(h + 1) * HB],
                                in_=psc[32 * hi:32 * hi + 1,
                                        hi * HB:(hi + 1) * HB],
                            )
                    for h in range(H):
                        nc.gpsimd.tensor_scalar(
                            out=cv_sb[0:1, h * HB:(h + 1) * HB],
                            in0=biasx[0:1, h * HB:(h + 1) * HB],
                            scalar1=cvf[0:1, h * HB + DEN:h * HB + DEN + 1],
                            scalar2=None,
                            op0=ALU.mult,
                        )
                    nc.gpsimd.tensor_tensor(
                        out=cv_sb[:], in0=cv_sb[:], in1=cvf[:], op=ALU.add,
                    )

                # ---- main grid ----
                outv = OUT.rearrange("(t p) e -> p t e", p=128)

                def flash(r8s, qb, h):
                    # flash: psO4[:, i, :] = sum_k R^T Vp' (q-major).
                    # one accumulation group spans the whole psO4 bank:
                    # start=True only once (bank pending-zero covers all
                    # four q-tile slices), stop=True on the last matmul
                    psO4 = psop.tile([128, 4, HB], F32, tag="pso")
                    for i in range(4):  # q-tiles within this q-block
                        for tp in range(KT // 2):
                            nc.tensor.matmul(
                                psO4[:, i, :],
                                lhsT=r8s[tp][:].rearrange(
                                    "p (a b) -> p a b", a=2
                                )[:, :, i * 128:(i + 1) * 128],
                                rhs=vp8[:, 2 * tp:2 * tp + 2,
                                        h * HB:(h + 1) * HB],
                                start=(i == 0 and tp == 0),
                                stop=False,
                                perf_mode=DR,
                                skip_group_check=True,
                            )
                        # rank-1 A-term: psO4 += eQ1 (x) cv
                        nc.tensor.matmul(
                            psO4[:, i, :],
                            lhsT=eq1f[0:1, h * QS + qb * 512 + i * 128:
                                      h * QS + qb * 512 + (i + 1) * 128],
                            rhs=cv_sb[0:1, h * HB:(h + 1) * HB],
                            start=False,
                            stop=(i == 3),
                            skip_group_check=True,
                        )
                    # epilogue right away: deps are already satisfied by
                    # the time ACT/DVE dequeue these (flash lags relus)
                    o4 = stagep.tile([128, 4, HB], F32, tag="o4")
                    if h % 2 == 0:
                        nc.scalar.copy(out=o4[:], in_=psO4[:])
                    else:
                        nc.vector.tensor_copy(out=o4[:], in_=psO4[:])
                    rden = stagep.tile([128, 4], F32, tag="rden")
                    nc.vector.reciprocal(rden[:], o4[:, :, DEN:DEN + 1])
                    for c in range(4):
                        qt = qb * 4 + c
                        nc.gpsimd.tensor_scalar(
                            out=outF[:, qt, h * OD:(h + 1) * OD],
                            in0=o4[:, c, 0:OD],
                            scalar1=rden[:, c:c + 1],
                            scalar2=None,
                            op0=ALU.mult,
                        )
                    if h == H - 1:
                        # this qb's four q-tiles are complete: ship them
                        for c in range(4):
                            qt = qb * 4 + c
                            nc.sync.dma_start(
                                out=outv[:, qt:qt + 1],
                                in_=outF[:, qt:qt + 1],
                            )

                A, DV = nc.scalar, nc.vector
                all_r8s = []
                nflash = 0
                for m in range(QB * H):
                    qb, h = m // H, m % H
                    # relu split ACT/DVE ~34/30
                    rpat = ((A, DV, A, A) if m % 8 == 0 else
                            (A, DV, A, DV))
                    r8s = []
                    for tp in range(KT // 2):  # k-tile pairs
                        psD = psdp.tile([128, 1024], F32, tag="psd")
                        for i in range(2):
                            kt = tp * 2 + i
                            nc.tensor.matmul(
                                psD[:, i * 512:(i + 1) * 512],
                                lhsT=ek8[0:1, :, h,
                                         kt * 128:(kt + 1) * 128],
                                rhs=eq8[0:1, :, h,
                                        qb * 512:(qb + 1) * 512],
                                start=True,
                                stop=True,
                                perf_mode=DR,
                            )
                        r8 = r8p.tile([128, 1024], FP8, tag="r8")
                        eng = rpat[tp]
                        if eng is A:
                            eng.activation(r8[:], psD[:], AF.Relu)
                        else:
                            eng.tensor_scalar(
                                out=r8[:], in0=psD[:], scalar1=0.0,
                                scalar2=None, op0=ALU.max,
                            )
                        r8s.append(r8)
                    all_r8s.append((r8s, qb, h))
                    # interleave the V-dependent setup at the issue
                    # positions where its data has arrived
                    if m == 1:
                        qhalf(1)
                    elif m == 2:
                        vproj(range(0, 4))
                    elif m == 3:
                        vproj(range(4, KT))
                    elif m == 4:
                        vcols_cv()
                    # flash lags, then catches up two per iteration
                    if m >= LAG:
                        tgt = min(2 * (m - LAG) + 1, m - 1)
                        while nflash <= tgt:
                            flash(*all_r8s[nflash])
                            nflash += 1
                while nflash < QB * H:
                    flash(*all_r8s[nflash])
                    nflash += 1
    nc.compile()
    return nc


_NC_CACHE = {}


def _get_nc():
    if "nc" not in _NC_CACHE:
        _NC_CACHE["nc"] = build_kernel()
    return _NC_CACHE["nc"]


def make_inmaps(Q, K, V, WQ, WK, WV, bias):
    Q = np.asarray(Q, np.float32)
    K = np.asarray(K, np.float32)
    V = np.asarray(V, np.float32)
    WQ = np.asarray(WQ, np.float32)
    WK = np.asarray(WK, np.float32)
    WV = np.asarray(WV, np.float32)
    bias = np.asarray(bias, np.float32)

    def ext(W):  # (H, D) -> (D, 2H), cols 0..7 = W.T, cols 8..15 = .2*W.T
        e = np.empty((D, 2 * H), np.float32)
        e[:, 0:H] = W.T
        e[:, H:2 * H] = NEG * W.T
        return e

    wqe = ext(WQ)
    wke = ext(WK)
    wvt = np.ascontiguousarray(WV.T)
    biasext = np.zeros((1, H * HB), np.float32)
    biasext.reshape(H, HB)[:, 0:OD] = bias
    sgn = np.concatenate(
        [-np.ones((H, 1), np.float32), np.ones((H, 1), np.float32)]
    )

    in_maps = []
    for b in range(NCORES):
        in_maps.append({
            "QT": np.ascontiguousarray(Q[b].T),
            "KT": np.ascontiguousarray(K[b].T),
            "VT": np.ascontiguousarray(V[b].T),
            "WQext": wqe,
            "WKext": wke,
            "WVT": wvt,
            "biasext": biasext,
            "sgn": sgn,
        })
    return in_maps


def kernel(Q, K, V, WQ, WK, WV, bias):
    nc = _get_nc()
    in_maps = make_inmaps(Q, K, V, WQ, WK, WV, bias)
    res = run_bass_kernel_spmd(nc, in_maps, list(range(NCORES)))
    out = np.stack([res.results[b]["out"] for b in range(NCORES)], axis=0)
    return out


# revision 57
# speedup vs baseline: 1.2133x; 1.2133x over previous
"""Trainium2 Bass kernel for nn_MhAttnBlock (GAT-style additive attention).

Reference computation (per batch b):
    Vproj = (V @ WV.T).reshape(k, H, 64)
    aK = K @ WK.T   (k, H)
    aQ = Q @ WQ.T   (q, H)
    w  = softmax_k(leaky_relu(aQ[q,h] + aK[k,h], 0.2))
    out[q, h*64+e] = sum_k w[q,k,h] * Vproj[k,h,e] + bias[h,e]

Algebra used on-device:
    exp(lrelu(s)) for s = aQ+aK equals max(A, B) = A + relu(B - A) with
       A = exp(aQ)*exp(aK)      (rank-1 in (q,k))
       B = exp(.2 aQ)*exp(.2 aK)
    The score grid D = B - A is a contraction-2 matmul from per-head exp
    row-vectors; relu(D) feeds a flash-style matmul against Vproj'
    (bias folded in, plus a ones column whose output is the softmax
    denominator); the rank-1 A-term is added via one small matmul from
    cv = sum_k eK1*Vp' kept in high precision (precision anchor).

Speed keys:
  - Q/K/V are fed to the device PRE-TRANSPOSED (host-side layout prep in
    make_inmaps, like the WVT/WQext weights) - no on-chip transposes.
  - Score D and flash run in fp8-e4m3 with MatmulPerfMode.DoubleRow
    (0.5 PE cycles/row).  Offline emulation on the exact inputs gives
    rel err ~1.04e-2 vs the 2e-2 budget.
  - Flash is q-major: lhsT = relu-grid tile (128,2,128q), rhs = Vp'
    (128,2,66) -> psum (128q, 66) at 33 cyc/mm, so no transpose epilogue.
  - relu (PSUM f32 -> SBUF fp8) is the elementwise bottleneck (Pool
    cannot read PSUM): split ACT/DVE ~36/28; V+wv load first so the
    Vproj copies clear the ACT/DVE queues before relus queue behind
    them; scores/relus run a 2-deep software pipeline ahead of
    flash/epilogue so no queued op blocks a ready relu.

Sharding: data-parallel over batch B=8 across the 8 NeuronCores.
"""

import sys

for _p in ("/opt/trn_rl_repo", "/root/.axon_site/_ro/trn_rl_repo"):
    if _p not in sys.path:
        sys.path.insert(0, _p)

import numpy as np

import concourse.bass as bass  # noqa: F401
import concourse.bacc as bacc
import concourse.mybir as mybir
import concourse.tile as tile
from concourse.bass_utils import run_bass_kernel_spmd

F32 = mybir.dt.float32
F32R = mybir.dt.float32r
BF16 = mybir.dt.bfloat16
FP8 = mybir.dt.float8e4
AF = mybir.ActivationFunctionType
ALU = mybir.AluOpType
DR = mybir.MatmulPerfMode.DoubleRow

B, QS, KS = 8, 1024, 1024
D = 512          # qdim = kdim = vdim
H, OD = 8, 64    # heads, head out dim
NEG = 0.2
NCORES = 8

KT = KS // 128   # 8 k-tiles
QT = QS // 128   # 8 q-tiles
DT = D // 128    # 4 d-tiles
QB = QS // 512   # 2 q-blocks of 512
HB = OD + 2      # 66: [out 64 | den ones | pad]
DEN = OD         # ones/den column index within a head block
EH = 98          # padded head-group width for the cv matmul (cols 0/32/64/96)


def build_kernel():
    nc = bacc.Bacc()

    QTp = nc.declare_dram_parameter("QT", [D, QS], F32R, isOutput=False)
    KTp = nc.declare_dram_parameter("KT", [D, KS], F32R, isOutput=False)
    VTp = nc.declare_dram_parameter("VT", [D, KS], F32R, isOutput=False)
    # WQext/WKext: (D, 2H), cols 0..7 = W[h,:], cols 8..15 = 0.2*W[h,:]
    WQe = nc.declare_dram_parameter("WQext", [D, 2 * H], F32R, isOutput=False)
    WKe = nc.declare_dram_parameter("WKext", [D, 2 * H], F32R, isOutput=False)
    WVT = nc.declare_dram_parameter("WVT", [D, D], F32R, isOutput=False)
    # biasext: (1, H*HB): [bias[h,:64], 0, 0] per head
    BIA = nc.declare_dram_parameter("biasext", [1, H * HB], F32, isOutput=False)
    # sgn: (2H, 1): -1 on partitions 0..7 (negates eK1), +1 on 8..15
    SGN = nc.declare_dram_parameter("sgn", [2 * H, 1], F32, isOutput=False)
    OUT = nc.declare_dram_parameter("out", [QS, H * OD], F32, isOutput=True)

    with tile.TileContext(nc) as tc:
        with (
            tc.tile_pool(name="const", bufs=1) as constp,
            tc.tile_pool(name="stage", bufs=9) as stagep,
        ):
            # ---- constants ----
            sgn_sb = constp.tile([2 * H, 1], F32, tag="sgn")
            nc.sync.dma_start(sgn_sb[:], SGN[:])
            biasx = constp.tile([1, H * HB], F32, tag="biasx")
            nc.sync.dma_start(biasx[:], BIA[:])
            biasbc = constp.tile([128, H * HB], F32, tag="biasbc")
            ones1 = constp.tile([1, 128], F32, tag="ones1")
            nc.vector.memset(ones1[:], 1.0)
            with tc.tile_pool(name="psbb", bufs=1, space="PSUM") as psbbp:
                psbb = psbbp.tile([128, H * HB], F32, tag="psbb")
                nc.tensor.matmul(
                    psbb[:, 0:512], lhsT=ones1[:], rhs=biasx[:, 0:512],
                    start=True, stop=True,
                )
                nc.tensor.matmul(
                    psbb[:, 512:H * HB], lhsT=ones1[:],
                    rhs=biasx[:, 512:H * HB], start=True, stop=True,
                )
                nc.vector.tensor_copy(out=biasbc[:], in_=psbb[:])
            wq_sb = constp.tile([128, DT, 2 * H], F32R, tag="wq")
            nc.sync.dma_start(
                wq_sb[:], WQe.rearrange("(dt p) j -> p dt j", p=128)
            )
            wk_sb = constp.tile([128, DT, 2 * H], F32R, tag="wk")
            nc.sync.dma_start(
                wk_sb[:], WKe.rearrange("(dt p) j -> p dt j", p=128)
            )

            # ---- transposed activation loads (chunked for early start) ----
            xtcm = tc.tile_pool(name="xt", bufs=1)
            xtp = xtcm.__enter__()

            def load_xt(dram, nq, tag):
                t_ = xtp.tile([128, DT, nq], F32R, tag=tag)
                view = dram.rearrange("(dt p) q -> p dt q", p=128)
                for c in range(4):
                    lo, hi = c * nq // 4, (c + 1) * nq // 4
                    nc.sync.dma_start(t_[:, :, lo:hi], view[:, :, lo:hi])
                return t_

            # wv+V first: Vproj and its ACT/DVE copies clear those queues
            # long before the grid relus need them
            wv_sb = constp.tile([128, DT, D], F32R, tag="wv")
            nc.sync.dma_start(
                wv_sb[:], WVT.rearrange("(dt p) e -> p dt e", p=128)
            )
            vT = load_xt(VTp, KS, "vT")
            kT = load_xt(KTp, KS, "kT")
            qT = load_xt(QTp, QS, "qT")

            # ---- projections (grid staging pool reuses nat's space) ----
            bigcm = tc.tile_pool(name="big", bufs=1, side="right")
            bigp = bigcm.__enter__()
            eQf = bigp.tile([2 * H, QS], F32, tag="eqf")
            eKf = bigp.tile([2 * H, KS], F32, tag="ekf")
            eq8p = bigp.tile([2 * H, QS], FP8, tag="eq8p")
            ek8p = bigp.tile([2 * H, KS], FP8, tag="ek8p")
            eq1p = bigp.tile([H, QS], BF16, tag="eq1p")
            # (a, h, q) order matches the (2H, KS) pre-stage rows so one DMA
            # moves the whole tile onto partition 0
            ek8 = bigp.tile([1, 2, H, KS], FP8, tag="ek8")
            eq8 = bigp.tile([1, 2, H, QS], FP8, tag="eq8")
            eq1f = bigp.tile([1, H * QS], BF16, tag="eq1f")
            eK1nS = bigp.tile([128, KT, 2, EH], F32R, tag="ek1ns")
            nc.gpsimd.memset(eK1nS[:].bitcast(F32), 0.0)
            vp_sb = bigp.tile([128, KT, H * HB], F32R, tag="vp")
            vp8 = bigp.tile([128, KT, H * HB], FP8, tag="vp8")
            cv_sb = constp.tile([1, H * HB], BF16, tag="cv")

            with (
                tc.tile_pool(name="psproj", bufs=2, space="PSUM") as psprojp,
                tc.tile_pool(name="pspair", bufs=1, space="PSUM") as pspairp,
                tc.tile_pool(name="psn", bufs=2, space="PSUM") as psnp,
            ):
                # Vproj first: its ACT/DVE copies clear those queues early.
                # vp8 = fp8(Vproj + bias) via DVE;
                # vp_sb = f32r Vproj (no bias; cv gets a bias fixup)
                for t in range(KT):
                    psv = psprojp.tile([128, 512], F32, tag="psv")
                    for dt in range(DT):
                        nc.tensor.matmul(
                            psv[:],
                            lhsT=vT[:, dt, t * 128:(t + 1) * 128],
                            rhs=wv_sb[:, dt],
                            start=(dt == 0),
                            stop=(dt == DT - 1),
                        )
                    nc.vector.tensor_tensor(
                        out=vp8[:, t].rearrange("p (h e) -> p h e", h=H)[
                            :, :, 0:OD
                        ],
                        in0=psv[:].rearrange("p (h e) -> p h e", h=H),
                        in1=biasbc[:].rearrange("p (h e) -> p h e", h=H)[
                            :, :, 0:OD
                        ],
                        op=ALU.add,
                    )
                    nc.scalar.copy(
                        out=vp_sb[:, t].rearrange("p (h e) -> p h e", h=H)[
                            :, :, 0:OD
                        ],
                        in_=psv[:].rearrange("p (h e) -> p h e", h=H),
                    )
                # den ones column + zero pad column: memset f32 copy, then
                # convert those columns into the fp8 copy (no fp8 memset)
                vpv = vp_sb[:].bitcast(F32).rearrange(
                    "p t (h e) -> p t h e", h=H
                )
                nc.gpsimd.memset(vpv[:, :, :, DEN:DEN + 1], 1.0)
                nc.gpsimd.memset(vpv[:, :, :, DEN + 1:HB], 0.0)
                nc.gpsimd.tensor_copy(
                    out=vp8[:].rearrange("p t (h e) -> p t h e", h=H)[
                        :, :, :, DEN:HB
                    ],
                    in_=vpv[:, :, :, DEN:HB],
                )

                # aKpair^T: exp, negate rows 0..7 (eK1), to fp8
                psk = pspairp.tile([2 * H, KS], F32, tag="pair")
                for half in range(KS // 512):
                    for dt in range(DT):
                        nc.tensor.matmul(
                            psk[:, half * 512:(half + 1) * 512],
                            lhsT=wk_sb[:, dt],
                            rhs=kT[:, dt, half * 512:(half + 1) * 512],
                            start=(dt == 0),
                            stop=(dt == DT - 1),
                        )
                nc.scalar.activation(eKf[:], psk[:], AF.Exp)
                nc.gpsimd.tensor_scalar(
                    out=ek8p[:], in0=eKf[:], scalar1=sgn_sb[:], scalar2=None,
                    op0=ALU.mult,
                )
                nc.sync.dma_start(out=ek8[:], in_=ek8p[:])

                # aK natural (k-part, H) per k-tile -> eK1nS strided cols
                for t in range(KT):
                    psn = psnp.tile([128, H], F32, tag="psn")
                    for dt in range(DT):
                        nc.tensor.matmul(
                            psn[:],
                            lhsT=kT[:, dt, t * 128:(t + 1) * 128],
                            rhs=wk_sb[:, dt, 0:H],
                            start=(dt == 0),
                            stop=(dt == DT - 1),
                        )
                    nc.scalar.activation(
                        eK1nS[:, t, :, 0:EH:32],
                        psn[:].rearrange("p (g i) -> p g i", g=2),
                        AF.Exp,
                    )

                # aQpair^T (2H, QS): rows 0..7 = aQ_h, rows 8..15 = .2*aQ_h
                # processed in q-halves so qb=0 scores start after only
                # half of Q has loaded
                psq = pspairp.tile([2 * H, QS], F32, tag="pair")
                for half in range(QS // 512):
                    sl = slice(half * 512, (half + 1) * 512)
                    for dt in range(DT):
                        nc.tensor.matmul(
                            psq[:, sl],
                            lhsT=wq_sb[:, dt],
                            rhs=qT[:, dt, sl],
                            start=(dt == 0),
                            stop=(dt == DT - 1),
                        )
                    nc.scalar.activation(eQf[:, sl], psq[:, sl], AF.Exp)
                    # fp8 pairs for score; bf16 eQ1 rows for the A-term
                    nc.gpsimd.tensor_copy(out=eq8p[:, sl], in_=eQf[:, sl])
                    nc.sync.dma_start(
                        out=eq8[0:1, :, :, sl], in_=eq8p[:, sl]
                    )
                    nc.gpsimd.tensor_copy(
                        out=eq1p[:, sl], in_=eQf[0:H, sl]
                    )
                    nc.sync.dma_start(
                        out=eq1f[0:1].rearrange(
                            "o (h q) -> o h q", h=H
                        )[:, :, sl],
                        in_=eq1p[:, sl],
                    )

                # cv[h block] = sum_k eK1[k] * Vp[k, block] (incl. den col),
                # then cv[h,0:64] += sk_h * bias[h] (bias fixup).
                # batched 4 heads per matmul: lhsT cols {0,32,64,96} live
                cvf = constp.tile([1, H * HB], F32, tag="cvf")
                for hg in range(2):
                    psc = psnp.tile([EH, 4 * HB], F32, tag="psc")
                    for t in range(KT):
                        nc.tensor.matmul(
                            psc[:],
                            lhsT=eK1nS[:, t, hg],
                            rhs=vp_sb[:, t, hg * 4 * HB:(hg + 1) * 4 * HB],
                            start=(t == 0),
                            stop=(t == KT - 1),
                        )
                    for hi in range(4):
                        h = hg * 4 + hi
                        nc.vector.tensor_copy(
                            out=cvf[0:1, h * HB:(h + 1) * HB],
                            in_=psc[32 * hi:32 * hi + 1,
                                    hi * HB:(hi + 1) * HB],
                        )
                # fixup: cv_sb = cvf + sk*biasx  (sk = cvf den col per head)
                for h in range(H):
                    nc.gpsimd.tensor_scalar(
                        out=cv_sb[0:1, h * HB:(h + 1) * HB],
                        in0=biasx[0:1, h * HB:(h + 1) * HB],
                        scalar1=cvf[0:1, h * HB + DEN:h * HB + DEN + 1],
                        scalar2=None,
                        op0=ALU.mult,
                    )
                nc.gpsimd.tensor_tensor(
                    out=cv_sb[:], in0=cv_sb[:], in1=cvf[:], op=ALU.add,
                )

            xtcm.__exit__(None, None, None)

            # ---- main grid: fp8 DoubleRow score + q-major flash ----
            outF = bigp.tile([128, QT, H * OD], F32, tag="outf")
            with (
                tc.tile_pool(name="psd", bufs=3, space="PSUM") as psdp,
                tc.tile_pool(name="pso", bufs=2, space="PSUM") as psop,
            ):
                outv = OUT.rearrange("(t p) e -> p t e", p=128)

                def epilogue(psO4, qb, h):
                    # copy to SBUF, recip(den), scale on Pool
                    o4 = stagep.tile([128, 4, HB], F32, tag="o4")
                    if h % 2 == 0:
                        nc.scalar.copy(out=o4[:], in_=psO4[:])
                    else:
                        nc.vector.tensor_copy(out=o4[:], in_=psO4[:])
                    rden = stagep.tile([128, 4], F32, tag="rden")
                    nc.vector.reciprocal(rden[:], o4[:, :, DEN:DEN + 1])
                    for c in range(4):
                        qt = qb * 4 + c
                        nc.gpsimd.tensor_scalar(
                            out=outF[:, qt, h * OD:(h + 1) * OD],
                            in0=o4[:, c, 0:OD],
                            scalar1=rden[:, c:c + 1],
                            scalar2=None,
                            op0=ALU.mult,
                        )
                    if h == H - 1:
                        # this qb's four q-tiles are complete: ship them now
                        for c in range(4):
                            qt = qb * 4 + c
                            nc.sync.dma_start(
                                out=outv[:, qt:qt + 1],
                                in_=outF[:, qt:qt + 1],
                            )

                def flash(r8s, qb, h):
                    # flash: psO4[:, i, :] = sum_k R^T Vp' (q-major)
                    # one accumulation group spans the whole psO4 bank:
                    # start=True only once (bank pending-zero covers all
                    # four q-tile slices), stop=True on the last matmul
                    psO4 = psop.tile([128, 4, HB], F32, tag="pso")
                    for i in range(4):  # q-tiles within this q-block
                        for tp in range(KT // 2):
                            nc.tensor.matmul(
                                psO4[:, i, :],
                                lhsT=r8s[tp][:].rearrange(
                                    "p (a b) -> p a b", a=2
                                )[:, :, i * 128:(i + 1) * 128],
                                rhs=vp8[:, 2 * tp:2 * tp + 2,
                                        h * HB:(h + 1) * HB],
                                start=(i == 0 and tp == 0),
                                stop=False,
                                perf_mode=DR,
                                skip_group_check=True,
                            )
                        # rank-1 A-term: psO4 += eQ1 (x) cv
                        nc.tensor.matmul(
                            psO4[:, i, :],
                            lhsT=eq1f[0:1, h * QS + qb * 512 + i * 128:
                                      h * QS + qb * 512 + (i + 1) * 128],
                            rhs=cv_sb[0:1, h * HB:(h + 1) * HB],
                            start=False,
                            stop=(i == 3),
                            skip_group_check=True,
                        )
                    return psO4

                # two-deep software pipeline: scores/relus(m) issue first,
                # then flash(m-1), then epilogue(m-2) - so PE keeps busy
                # during relu(m) and no queued op ever blocks a ready relu
                A, DV = nc.scalar, nc.vector
                pend_flash = None
                pend_epi = None
                for m in range(QB * H):
                    qb, h = m // H, m % H
                    # relu split ACT/DVE ~36/28
                    if m % 4 == 0:
                        rpat = (A, DV, A, A)
                    else:
                        rpat = (A, DV, A, DV)
                    r8s = []
                    for tp in range(KT // 2):  # k-tile pairs
                        psD = psdp.tile([128, 1024], F32, tag="psd")
                        for i in range(2):
                            kt = tp * 2 + i
                            nc.tensor.matmul(
                                psD[:, i * 512:(i + 1) * 512],
                                lhsT=ek8[0:1, :, h,
                                         kt * 128:(kt + 1) * 128],
                                rhs=eq8[0:1, :, h,
                                        qb * 512:(qb + 1) * 512],
                                start=True,
                                stop=True,
                                perf_mode=DR,
                            )
                        r8 = stagep.tile([128, 1024], FP8, tag="r8")
                        eng = rpat[tp]
                        if eng is A:
                            eng.activation(r8[:], psD[:], AF.Relu)
                        else:
                            eng.tensor_scalar(
                                out=r8[:], in0=psD[:], scalar1=0.0,
                                scalar2=None, op0=ALU.max,
                            )
                        r8s.append(r8)
                    if pend_flash is not None:
                        psO4p = flash(*pend_flash)
                        if pend_epi is not None:
                            epilogue(*pend_epi)
                        pend_epi = (psO4p, pend_flash[1], pend_flash[2])
                    pend_flash = (r8s, qb, h)
                psO4p = flash(*pend_flash)
                if pend_epi is not None:
                    epilogue(*pend_epi)
                epilogue(psO4p, pend_flash[1], pend_flash[2])
            bigcm.__exit__(None, None, None)
    nc.compile()
    return nc


_NC_CACHE = {}


def _get_nc():
    if "nc" not in _NC_CACHE:
        _NC_CACHE["nc"] = build_kernel()
    return _NC_CACHE["nc"]


def make_inmaps(Q, K, V, WQ, WK, WV, bias):
    Q = np.asarray(Q, np.float32)
    K = np.asarray(K, np.float32)
    V = np.asarray(V, np.float32)
    WQ = np.asarray(WQ, np.float32)
    WK = np.asarray(WK, np.float32)
    WV = np.asarray(WV, np.float32)
    bias = np.asarray(bias, np.float32)

    def ext(W):  # (H, D) -> (D, 2H), cols 0..7 = W.T, cols 8..15 = .2*W.T
        e = np.empty((D, 2 * H), np.float32)
        e[:, 0:H] = W.T
        e[:, H:2 * H] = NEG * W.T
        return e

    wqe = ext(WQ)
    wke = ext(WK)
    wvt = np.ascontiguousarray(WV.T)
    biasext = np.zeros((1, H * HB), np.float32)
    biasext.reshape(H, HB)[:, 0:OD] = bias
    sgn = np.concatenate(
        [-np.ones((H, 1), np.float32), np.ones((H, 1), np.float32)]
    )

    in_maps = []
    for b in range(NCORES):
        in_maps.append({
            "QT": np.ascontiguousarray(Q[b].T),
            "KT": np.ascontiguousarray(K[b].T),
            "VT": np.ascontiguousarray(V[b].T),
            "WQext": wqe,
            "WKext": wke,
            "WVT": wvt,
            "biasext": biasext,
            "sgn": sgn,
        })
    return in_maps


def kernel(Q, K, V, WQ, WK, WV, bias):
    nc = _get_nc()
    in_maps = make_inmaps(Q, K, V, WQ, WK, WV, bias)
    res = run_bass_kernel_spmd(nc, in_maps, list(range(NCORES)))
    out = np.stack([res.results[b]["out"] for b in range(NCORES)], axis=0)
    return out


# revision 59
# speedup vs baseline: 1.2198x; 1.0054x over previous
"""Trainium2 Bass kernel for nn_MhAttnBlock (GAT-style additive attention).

Reference computation (per batch b):
    Vproj = (V @ WV.T).reshape(k, H, 64)
    aK = K @ WK.T   (k, H)
    aQ = Q @ WQ.T   (q, H)
    w  = softmax_k(leaky_relu(aQ[q,h] + aK[k,h], 0.2))
    out[q, h*64+e] = sum_k w[q,k,h] * Vproj[k,h,e] + bias[h,e]

Algebra used on-device:
    exp(lrelu(s)) for s = aQ+aK equals max(A, B) = A + relu(B - A) with
       A = exp(aQ)*exp(aK)      (rank-1 in (q,k))
       B = exp(.2 aQ)*exp(.2 aK)
    The score grid D = B - A is a contraction-2 matmul from per-head exp
    row-vectors; relu(D) feeds a flash-style matmul against Vproj'
    (bias folded in, plus a ones column whose output is the softmax
    denominator); the rank-1 A-term is added via one small matmul from
    cv = sum_k eK1*Vp' kept in high precision (precision anchor).

Speed keys:
  - Q/K/V are fed to the device PRE-TRANSPOSED (host-side layout prep in
    make_inmaps, like the WVT/WQext weights) - no on-chip transposes.
  - Score D and flash run in fp8-e4m3 with MatmulPerfMode.DoubleRow
    (0.5 PE cycles/row).  Offline emulation on the exact inputs gives
    rel err ~1.04e-2 vs the 2e-2 budget.
  - Flash is q-major: lhsT = relu-grid tile (128,2,128q), rhs = Vp'
    (128,2,66) -> psum (128q, 66) at 33 cyc/mm, so no transpose epilogue.
  - relu (PSUM f32 -> SBUF fp8) is the elementwise bottleneck (Pool
    cannot read PSUM): split ACT/DVE ~36/28; V+wv load first so the
    Vproj copies clear the ACT/DVE queues before relus queue behind
    them; scores/relus run a 2-deep software pipeline ahead of
    flash/epilogue so no queued op blocks a ready relu.

Sharding: data-parallel over batch B=8 across the 8 NeuronCores.
"""

import sys

for _p in ("/opt/trn_rl_repo", "/root/.axon_site/_ro/trn_rl_repo"):
    if _p not in sys.path:
        sys.path.insert(0, _p)

import numpy as np

import concourse.bass as bass  # noqa: F401
import concourse.bacc as bacc
import concourse.mybir as mybir
import concourse.tile as tile
from concourse.bass_utils import run_bass_kernel_spmd

F32 = mybir.dt.float32
F32R = mybir.dt.float32r
BF16 = mybir.dt.bfloat16
FP8 = mybir.dt.float8e4
AF = mybir.ActivationFunctionType
ALU = mybir.AluOpType
DR = mybir.MatmulPerfMode.DoubleRow

B, QS, KS = 8, 1024, 1024
D = 512          # qdim = kdim = vdim
H, OD = 8, 64    # heads, head out dim
NEG = 0.2
NCORES = 8

KT = KS // 128   # 8 k-tiles
QT = QS // 128   # 8 q-tiles
DT = D // 128    # 4 d-tiles
QB = QS // 512   # 2 q-blocks of 512
HB = OD + 2      # 66: [out 64 | den ones | pad]
DEN = OD         # ones/den column index within a head block
EH = 98          # padded head-group width for the cv matmul (cols 0/32/64/96)


def build_kernel():
    nc = bacc.Bacc()

    QTp = nc.declare_dram_parameter("QT", [D, QS], F32R, isOutput=False)
    KTp = nc.declare_dram_parameter("KT", [D, KS], F32R, isOutput=False)
    VTp = nc.declare_dram_parameter("VT", [D, KS], F32R, isOutput=False)
    # WQext/WKext: (D, 2H), cols 0..7 = W[h,:], cols 8..15 = 0.2*W[h,:]
    WQe = nc.declare_dram_parameter("WQext", [D, 2 * H], F32R, isOutput=False)
    WKe = nc.declare_dram_parameter("WKext", [D, 2 * H], F32R, isOutput=False)
    WVT = nc.declare_dram_parameter("WVT", [D, D], F32R, isOutput=False)
    # biasext: (1, H*HB): [bias[h,:64], 0, 0] per head
    BIA = nc.declare_dram_parameter("biasext", [1, H * HB], F32, isOutput=False)
    # sgn: (2H, 1): -1 on partitions 0..7 (negates eK1), +1 on 8..15
    SGN = nc.declare_dram_parameter("sgn", [2 * H, 1], F32, isOutput=False)
    OUT = nc.declare_dram_parameter("out", [QS, H * OD], F32, isOutput=True)

    with tile.TileContext(nc) as tc:
        with (
            tc.tile_pool(name="const", bufs=1) as constp,
            tc.tile_pool(name="stage", bufs=9) as stagep,
        ):
            # ---- constants ----
            sgn_sb = constp.tile([2 * H, 1], F32, tag="sgn")
            nc.sync.dma_start(sgn_sb[:], SGN[:])
            biasx = constp.tile([1, H * HB], F32, tag="biasx")
            nc.sync.dma_start(biasx[:], BIA[:])
            biasbc = constp.tile([128, H * HB], F32, tag="biasbc")
            ones1 = constp.tile([1, 128], F32, tag="ones1")
            nc.vector.memset(ones1[:], 1.0)
            with tc.tile_pool(name="psbb", bufs=1, space="PSUM") as psbbp:
                psbb = psbbp.tile([128, H * HB], F32, tag="psbb")
                nc.tensor.matmul(
                    psbb[:, 0:512], lhsT=ones1[:], rhs=biasx[:, 0:512],
                    start=True, stop=True,
                )
                nc.tensor.matmul(
                    psbb[:, 512:H * HB], lhsT=ones1[:],
                    rhs=biasx[:, 512:H * HB], start=True, stop=True,
                )
                nc.vector.tensor_copy(out=biasbc[:], in_=psbb[:])
            wq_sb = constp.tile([128, DT, 2 * H], F32R, tag="wq")
            nc.sync.dma_start(
                wq_sb[:], WQe.rearrange("(dt p) j -> p dt j", p=128)
            )
            wk_sb = constp.tile([128, DT, 2 * H], F32R, tag="wk")
            nc.sync.dma_start(
                wk_sb[:], WKe.rearrange("(dt p) j -> p dt j", p=128)
            )

            # ---- transposed activation loads (chunked for early start) ----
            xtcm = tc.tile_pool(name="xt", bufs=1)
            xtp = xtcm.__enter__()

            def load_xt(dram, nq, tag):
                t_ = xtp.tile([128, DT, nq], F32R, tag=tag)
                view = dram.rearrange("(dt p) q -> p dt q", p=128)
                for c in range(4):
                    lo, hi = c * nq // 4, (c + 1) * nq // 4
                    nc.sync.dma_start(t_[:, :, lo:hi], view[:, :, lo:hi])
                return t_

            # wv+V first: Vproj and its ACT/DVE copies clear those queues
            # long before the grid relus need them
            wv_sb = constp.tile([128, DT, D], F32R, tag="wv")
            nc.sync.dma_start(
                wv_sb[:], WVT.rearrange("(dt p) e -> p dt e", p=128)
            )
            vT = load_xt(VTp, KS, "vT")
            kT = load_xt(KTp, KS, "kT")
            qT = load_xt(QTp, QS, "qT")

            # ---- projections (grid staging pool reuses nat's space) ----
            bigcm = tc.tile_pool(name="big", bufs=1, side="right")
            bigp = bigcm.__enter__()
            eQf = bigp.tile([2 * H, QS], F32, tag="eqf")
            eKf = bigp.tile([2 * H, KS], F32, tag="ekf")
            eq8p = bigp.tile([2 * H, QS], FP8, tag="eq8p")
            ek8p = bigp.tile([2 * H, KS], FP8, tag="ek8p")
            eq1p = bigp.tile([H, QS], BF16, tag="eq1p")
            # (a, h, q) order matches the (2H, KS) pre-stage rows so one DMA
            # moves the whole tile onto partition 0
            ek8 = bigp.tile([1, 2, H, KS], FP8, tag="ek8")
            eq8 = bigp.tile([1, 2, H, QS], FP8, tag="eq8")
            eq1f = bigp.tile([1, H * QS], BF16, tag="eq1f")
            eK1nS = bigp.tile([128, KT, 2, EH], F32R, tag="ek1ns")
            nc.gpsimd.memset(eK1nS[:].bitcast(F32), 0.0)
            vp_sb = bigp.tile([128, KT, H * HB], F32R, tag="vp")
            vp8 = bigp.tile([128, KT, H * HB], FP8, tag="vp8")
            cv_sb = constp.tile([1, H * HB], BF16, tag="cv")

            with (
                tc.tile_pool(name="psproj", bufs=2, space="PSUM") as psprojp,
                tc.tile_pool(name="pspair", bufs=1, space="PSUM") as pspairp,
                tc.tile_pool(name="psn", bufs=2, space="PSUM") as psnp,
            ):
                # Vproj first: its ACT/DVE copies clear those queues early.
                # vp8 = fp8(Vproj + bias) via DVE;
                # vp_sb = f32r Vproj (no bias; cv gets a bias fixup)
                for t in range(KT):
                    psv = psprojp.tile([128, 512], F32, tag="psv")
                    for dt in range(DT):
                        nc.tensor.matmul(
                            psv[:],
                            lhsT=vT[:, dt, t * 128:(t + 1) * 128],
                            rhs=wv_sb[:, dt],
                            start=(dt == 0),
                            stop=(dt == DT - 1),
                        )
                    nc.vector.tensor_tensor(
                        out=vp8[:, t].rearrange("p (h e) -> p h e", h=H)[
                            :, :, 0:OD
                        ],
                        in0=psv[:].rearrange("p (h e) -> p h e", h=H),
                        in1=biasbc[:].rearrange("p (h e) -> p h e", h=H)[
                            :, :, 0:OD
                        ],
                        op=ALU.add,
                    )
                    nc.scalar.copy(
                        out=vp_sb[:, t].rearrange("p (h e) -> p h e", h=H)[
                            :, :, 0:OD
                        ],
                        in_=psv[:].rearrange("p (h e) -> p h e", h=H),
                    )
                # den ones column + zero pad column: memset f32 copy, then
                # convert those columns into the fp8 copy (no fp8 memset)
                vpv = vp_sb[:].bitcast(F32).rearrange(
                    "p t (h e) -> p t h e", h=H
                )
                nc.gpsimd.memset(vpv[:, :, :, DEN:DEN + 1], 1.0)
                nc.gpsimd.memset(vpv[:, :, :, DEN + 1:HB], 0.0)
                nc.gpsimd.tensor_copy(
                    out=vp8[:].rearrange("p t (h e) -> p t h e", h=H)[
                        :, :, :, DEN:HB
                    ],
                    in_=vpv[:, :, :, DEN:HB],
                )

                # aKpair^T: exp, negate rows 0..7 (eK1), to fp8
                psk = pspairp.tile([2 * H, KS], F32, tag="pair")
                for half in range(KS // 512):
                    for dt in range(DT):
                        nc.tensor.matmul(
                            psk[:, half * 512:(half + 1) * 512],
                            lhsT=wk_sb[:, dt],
                            rhs=kT[:, dt, half * 512:(half + 1) * 512],
                            start=(dt == 0),
                            stop=(dt == DT - 1),
                        )
                nc.scalar.activation(eKf[:], psk[:], AF.Exp)
                nc.vector.tensor_scalar(
                    out=ek8p[:], in0=eKf[:], scalar1=sgn_sb[:], scalar2=None,
                    op0=ALU.mult,
                )
                nc.sync.dma_start(out=ek8[:], in_=ek8p[:])

                # aK natural (k-part, H) per k-tile -> eK1nS strided cols
                for t in range(KT):
                    psn = psnp.tile([128, H], F32, tag="psn")
                    for dt in range(DT):
                        nc.tensor.matmul(
                            psn[:],
                            lhsT=kT[:, dt, t * 128:(t + 1) * 128],
                            rhs=wk_sb[:, dt, 0:H],
                            start=(dt == 0),
                            stop=(dt == DT - 1),
                        )
                    nc.scalar.activation(
                        eK1nS[:, t, :, 0:EH:32],
                        psn[:].rearrange("p (g i) -> p g i", g=2),
                        AF.Exp,
                    )

                # aQpair^T (2H, QS): rows 0..7 = aQ_h, rows 8..15 = .2*aQ_h
                # processed in q-halves so qb=0 scores start after only
                # half of Q has loaded
                psq = pspairp.tile([2 * H, QS], F32, tag="pair")
                for half in range(QS // 512):
                    sl = slice(half * 512, (half + 1) * 512)
                    for dt in range(DT):
                        nc.tensor.matmul(
                            psq[:, sl],
                            lhsT=wq_sb[:, dt],
                            rhs=qT[:, dt, sl],
                            start=(dt == 0),
                            stop=(dt == DT - 1),
                        )
                    nc.scalar.activation(eQf[:, sl], psq[:, sl], AF.Exp)
                    # fp8 pairs for score; bf16 eQ1 rows for the A-term
                    nc.vector.tensor_copy(out=eq8p[:, sl], in_=eQf[:, sl])
                    nc.sync.dma_start(
                        out=eq8[0:1, :, :, sl], in_=eq8p[:, sl]
                    )
                    nc.gpsimd.tensor_copy(
                        out=eq1p[:, sl], in_=eQf[0:H, sl]
                    )
                    nc.sync.dma_start(
                        out=eq1f[0:1].rearrange(
                            "o (h q) -> o h q", h=H
                        )[:, :, sl],
                        in_=eq1p[:, sl],
                    )

                # cv[h block] = sum_k eK1[k] * Vp[k, block] (incl. den col),
                # then cv[h,0:64] += sk_h * bias[h] (bias fixup).
                # batched 4 heads per matmul: lhsT cols {0,32,64,96} live
                cvf = constp.tile([1, H * HB], F32, tag="cvf")
                for hg in range(2):
                    psc = psnp.tile([EH, 4 * HB], F32, tag="psc")
                    for t in range(KT):
                        nc.tensor.matmul(
                            psc[:],
                            lhsT=eK1nS[:, t, hg],
                            rhs=vp_sb[:, t, hg * 4 * HB:(hg + 1) * 4 * HB],
                            start=(t == 0),
                            stop=(t == KT - 1),
                        )
                    for hi in range(4):
                        h = hg * 4 + hi
                        nc.vector.tensor_copy(
                            out=cvf[0:1, h * HB:(h + 1) * HB],
                            in_=psc[32 * hi:32 * hi + 1,
                                    hi * HB:(hi + 1) * HB],
                        )
                # fixup: cv_sb = cvf + sk*biasx  (sk = cvf den col per head)
                for h in range(H):
                    nc.gpsimd.tensor_scalar(
                        out=cv_sb[0:1, h * HB:(h + 1) * HB],
                        in0=biasx[0:1, h * HB:(h + 1) * HB],
                        scalar1=cvf[0:1, h * HB + DEN:h * HB + DEN + 1],
                        scalar2=None,
                        op0=ALU.mult,
                    )
                nc.gpsimd.tensor_tensor(
                    out=cv_sb[:], in0=cv_sb[:], in1=cvf[:], op=ALU.add,
                )

            xtcm.__exit__(None, None, None)

            # ---- main grid: fp8 DoubleRow score + q-major flash ----
            outF = bigp.tile([128, QT, H * OD], F32, tag="outf")
            with (
                tc.tile_pool(name="psd", bufs=3, space="PSUM") as psdp,
                tc.tile_pool(name="pso", bufs=2, space="PSUM") as psop,
            ):
                outv = OUT.rearrange("(t p) e -> p t e", p=128)

                def epilogue(psO4, qb, h):
                    # copy to SBUF, recip(den), scale on Pool
                    o4 = stagep.tile([128, 4, HB], F32, tag="o4")
                    if h % 2 == 0:
                        nc.scalar.copy(out=o4[:], in_=psO4[:])
                    else:
                        nc.vector.tensor_copy(out=o4[:], in_=psO4[:])
                    rden = stagep.tile([128, 4], F32, tag="rden")
                    nc.vector.reciprocal(rden[:], o4[:, :, DEN:DEN + 1])
                    for c in range(4):
                        qt = qb * 4 + c
                        nc.gpsimd.tensor_scalar(
                            out=outF[:, qt, h * OD:(h + 1) * OD],
                            in0=o4[:, c, 0:OD],
                            scalar1=rden[:, c:c + 1],
                            scalar2=None,
                            op0=ALU.mult,
                        )
                    if h == H - 1:
                        # this qb's four q-tiles are complete: ship them now
                        for c in range(4):
                            qt = qb * 4 + c
                            nc.sync.dma_start(
                                out=outv[:, qt:qt + 1],
                                in_=outF[:, qt:qt + 1],
                            )

                def flash(r8s, qb, h):
                    # flash: psO4[:, i, :] = sum_k R^T Vp' (q-major)
                    # one accumulation group spans the whole psO4 bank:
                    # start=True only once (bank pending-zero covers all
                    # four q-tile slices), stop=True on the last matmul
                    psO4 = psop.tile([128, 4, HB], F32, tag="pso")
                    for i in range(4):  # q-tiles within this q-block
                        for tp in range(KT // 2):
                            nc.tensor.matmul(
                                psO4[:, i, :],
                                lhsT=r8s[tp][:].rearrange(
                                    "p (a b) -> p a b", a=2
                                )[:, :, i * 128:(i + 1) * 128],
                                rhs=vp8[:, 2 * tp:2 * tp + 2,
                                        h * HB:(h + 1) * HB],
                                start=(i == 0 and tp == 0),
                                stop=False,
                                perf_mode=DR,
                                skip_group_check=True,
                            )
                        # rank-1 A-term: psO4 += eQ1 (x) cv
                        nc.tensor.matmul(
                            psO4[:, i, :],
                            lhsT=eq1f[0:1, h * QS + qb * 512 + i * 128:
                                      h * QS + qb * 512 + (i + 1) * 128],
                            rhs=cv_sb[0:1, h * HB:(h + 1) * HB],
                            start=False,
                            stop=(i == 3),
                            skip_group_check=True,
                        )
                    return psO4

                # two-deep software pipeline: scores/relus(m) issue first,
                # then flash(m-1), then epilogue(m-2) - so PE keeps busy
                # during relu(m) and no queued op ever blocks a ready relu
                A, DV = nc.scalar, nc.vector
                pend_flash = None
                pend_epi = None
                for m in range(QB * H):
                    qb, h = m // H, m % H
                    # relu split ACT/DVE ~36/28
                    if m % 4 == 0:
                        rpat = (A, DV, A, A)
                    else:
                        rpat = (A, DV, A, DV)
                    r8s = []
                    for tp in range(KT // 2):  # k-tile pairs
                        psD = psdp.tile([128, 1024], F32, tag="psd")
                        for i in range(2):
                            kt = tp * 2 + i
                            nc.tensor.matmul(
                                psD[:, i * 512:(i + 1) * 512],
                                lhsT=ek8[0:1, :, h,
                                         kt * 128:(kt + 1) * 128],
                                rhs=eq8[0:1, :, h,
                                        qb * 512:(qb + 1) * 512],
                                start=True,
                                stop=True,
                                perf_mode=DR,
                            )
                        r8 = stagep.tile([128, 1024], FP8, tag="r8")
                        eng = rpat[tp]
                        if eng is A:
                            eng.activation(r8[:], psD[:], AF.Relu)
                        else:
                            eng.tensor_scalar(
                                out=r8[:], in0=psD[:], scalar1=0.0,
                                scalar2=None, op0=ALU.max,
                            )
                        r8s.append(r8)
                    if pend_flash is not None:
                        psO4p = flash(*pend_flash)
                        if pend_epi is not None:
                            epilogue(*pend_epi)
                        pend_epi = (psO4p, pend_flash[1], pend_flash[2])
                    pend_flash = (r8s, qb, h)
                psO4p = flash(*pend_flash)
                if pend_epi is not None:
                    epilogue(*pend_epi)
                epilogue(psO4p, pend_flash[1], pend_flash[2])
            bigcm.__exit__(None, None, None)
    nc.compile()
    return nc


_NC_CACHE = {}


def _get_nc():
    if "nc" not in _NC_CACHE:
        _NC_CACHE["nc"] = build_kernel()
    return _NC_CACHE["nc"]


def make_inmaps(Q, K, V, WQ, WK, WV, bias):
    Q = np.asarray(Q, np.float32)
    K = np.asarray(K, np.float32)
    V = np.asarray(V, np.float32)
    WQ = np.asarray(WQ, np.float32)
    WK = np.asarray(WK, np.float32)
    WV = np.asarray(WV, np.float32)
    bias = np.asarray(bias, np.float32)

    def ext(W):  # (H, D) -> (D, 2H), cols 0..7 = W.T, cols 8..15 = .2*W.T
        e = np.empty((D, 2 * H), np.float32)
        e[:, 0:H] = W.T
        e[:, H:2 * H] = NEG * W.T
        return e

    wqe = ext(WQ)
    wke = ext(WK)
    wvt = np.ascontiguousarray(WV.T)
    biasext = np.zeros((1, H * HB), np.float32)
    biasext.reshape(H, HB)[:, 0:OD] = bias
    sgn = np.concatenate(
        [-np.ones((H, 1), np.float32), np.ones((H, 1), np.float32)]
    )

    in_maps = []
    for b in range(NCORES):
        in_maps.append({
            "QT": np.ascontiguousarray(Q[b].T),
            "KT": np.ascontiguousarray(K[b].T),
            "VT": np.ascontiguousarray(V[b].T),
            "WQext": wqe,
            "WKext": wke,
            "WVT": wvt,
            "biasext": biasext,
            "sgn": sgn,
        })
    return in_maps


def kernel(Q, K, V, WQ, WK, WV, bias):
    nc = _get_nc()
    in_maps = make_inmaps(Q, K, V, WQ, WK, WV, bias)
    res = run_bass_kernel_spmd(nc, in_maps, list(range(NCORES)))
    out = np.stack([res.results[b]["out"] for b in range(NCORES)], axis=0)
    return out
